# revision 15
# baseline (speedup 1.0000x reference)
"""AttentiveAggregation (segment softmax-pool) Trainium2 kernel, v2.

Math (per graph g): out_g = sum_v alpha_v H_v,  alpha = softmax_g(e),
  e_v = w_score . tanh(W_proj @ H_v + b_proj).

Device/host split (per 128-node tile, 8 cores x 16 blocks of 128 segments):
 * projT = W_proj @ H^T as one PE matmul per 512 nodes with the constant
   W_proj^T stationary; b_proj is a native per-partition ACT bias and the
   tanh runs on ACT in 512-wide chunks -> tanh_g [k, nodes] f16.
 * e-score on PE: lhsT = tanh tile [k, 128], rhs = w_score column -> e
   lands as [128 nodes, 1] PSUM, exactly the layout exp + the one-hot
   multiply need.  exp (with the global shift C = ||w_score||_1 plus an
   f16-range prescale) on ACT -> a [128, GRP] f32 in SBUF.
 * narrow one-hot on DVE: oh[n, j] = (seg_rel(n) == j) * a_n over a
   [128, W] window (W = max segments any tile spans, >= 8); seg_rel is
   tile-relative and streamed as data, so the program stays SPMD-uniform.
 * flipped segment matmul on PE: lhsT = H tile [node, d], rhs = oh ->
   partial [128 d, W] written at the tile's fixed PSUM slot (t*W).
   Partials and the a values are DMA'd out; the HOST does the final
   scatter-add over per-tile segment bases, the denominator bincount,
   and the division (cheap: ~16k x 128).
 * Streams: ht (H^T per tile) on the ACT HWDGE queue, hseg (H per tile)
   + bl + partials on SP, a-values on the Pool SWDGE queue.
"""

import math

import numpy as np

P = 128                    # partitions / tile node count / D / HS
D = 128
G_SEGS = 16384
SEGS_PER_BLK = 128
NBLK_TOT = G_SEGS // SEGS_PER_BLK   # 128 global blocks
N_CORES = 8
NBLK = NBLK_TOT // N_CORES          # 16 blocks per core
GRP = 16                   # tiles per DMA group / pipeline stage

# alpha values are prescaled by exp(PRESCALE_LN) inside the exp bias so the
# per-segment weights stay in f16 normal range; numerator and denominator
# scale together so the final division cancels it exactly.
PRESCALE_LN = 14 * math.log(2.0)

CFG = {
    "w_min": 8,            # min one-hot width (keeps partial-DMA descs >=512B)
    "bufs_ht": 4,
    "bufs_hseg": 6,
    "bufs_tanh": 3,
    "bufs_oh": 36,
    "bufs_a": 4,
    "pp_proj": 2,
    "pp_e": 2,
    "pp_seg": 2,
    "tanh_chunk": 1024,
}


def _build_program(t_max, w, c_shift, cfg):
    import concourse.bacc as bacc
    import concourse.mybir as mybir
    import concourse.tile as tile

    f32 = mybir.dt.float32
    f16 = mybir.dt.float16
    NG = t_max // GRP
    NITER = NBLK * NG          # pipeline iterations (one per tile group)

    # fit the three PSUM pools into the 8 banks (2KB each per partition)
    cfg = dict(cfg)

    def banks():
        bp = -(-4 * cfg["tanh_chunk"] // 2048) * cfg["pp_proj"]
        be = cfg["pp_e"]
        bs = -(-GRP // (512 // w)) * cfg["pp_seg"]
        return bp + be + bs

    for knob, val in (("tanh_chunk", 512), ("pp_seg", 1), ("pp_proj", 1)):
        if banks() > 8:
            cfg[knob] = val
    assert banks() <= 8, f"PSUM over budget: {banks()} banks (w={w})"

    nc = bacc.Bacc(None, target_bir_lowering=False)
    ht_d = nc.dram_tensor("ht", [NBLK, NG, P, GRP * P], f16,
                          kind="ExternalInput")
    hseg_d = nc.dram_tensor("hseg", [NBLK, NG, P, GRP * P], f16,
                            kind="ExternalInput")
    bl_d = nc.dram_tensor("bl", [NBLK, P, t_max], f32, kind="ExternalInput")
    wt_d = nc.dram_tensor("wt", [P, D], f16, kind="ExternalInput")
    wb_d = nc.dram_tensor("wb", [P, 1], f16, kind="ExternalInput")
    bc_d = nc.dram_tensor("bc", [P, 1], f32, kind="ExternalInput")
    iota_d = nc.dram_tensor("iota", [P, w], f16, kind="ExternalInput")
    part_d = nc.dram_tensor("part", [NBLK, NG, P, GRP * w], f32,
                            kind="ExternalOutput")
    av_d = nc.dram_tensor("av", [NBLK, NG, P, GRP], f32,
                          kind="ExternalOutput")

    with tile.TileContext(nc) as tc:
        with (
            tc.tile_pool(name="const", bufs=1) as constp,
            tc.tile_pool(name="htp", bufs=cfg["bufs_ht"]) as htp,
            tc.tile_pool(name="hsegp", bufs=cfg["bufs_hseg"]) as hsegp,
            tc.tile_pool(name="tanhp", bufs=cfg["bufs_tanh"]) as tanhp,
            tc.tile_pool(name="ohp", bufs=cfg["bufs_oh"]) as ohp,
            tc.tile_pool(name="ap_", bufs=cfg["bufs_a"]) as ap_,
            tc.tile_pool(name="blp", bufs=4) as blp,
            tc.tile_pool(name="partp", bufs=3) as partp,
            tc.tile_pool(name="ppp", bufs=cfg["pp_proj"],
                         space="PSUM") as ppp,
            tc.tile_pool(name="pep", bufs=cfg["pp_e"], space="PSUM") as pep,
            tc.tile_pool(name="psp", bufs=cfg["pp_seg"], space="PSUM") as psp,
        ):
            wt_sb = constp.tile([P, D], f16)
            nc.sync.dma_start(wt_sb[:], wt_d[:])
            wb_sb = constp.tile([P, 1], f16)
            nc.sync.dma_start(wb_sb[:], wb_d[:])
            bc_sb = constp.tile([P, 1], f32)
            nc.sync.dma_start(bc_sb[:], bc_d[:])
            iota_sb = constp.tile([P, w], f16)
            nc.sync.dma_start(iota_sb[:], iota_d[:])
            negc_sb = constp.tile([P, 1], f32)
            nc.gpsimd.memset(negc_sb[:], -float(c_shift) + PRESCALE_LN)

            # per-iteration state carried across pipeline stages
            st = [None] * NITER
            bl_tiles = [None] * NBLK

            def stage_in(it):
                i, g = divmod(it, NG)
                if g == 0:
                    bl_sb = blp.tile([P, t_max], f32)
                    nc.sync.dma_start(bl_sb[:], bl_d[i])
                    bl_tiles[i] = bl_sb
                ht_sb = htp.tile([P, GRP * P], f16)
                nc.scalar.dma_start(ht_sb[:], ht_d[i, g])
                hseg_sb = hsegp.tile([P, GRP, P], f16)
                nc.sync.dma_start(
                    hseg_sb[:],
                    hseg_d[i, g].rearrange("p (t c) -> p t c", t=GRP))
                st[it] = {"ht": ht_sb, "hseg": hseg_sb, "bl": bl_tiles[i],
                          "i": i, "g": g}

            def stage_proj(it):
                s = st[it]
                tanh_g = tanhp.tile([P, GRP * P], f16, tag="tanh")
                ck = cfg["tanh_chunk"]
                for c in range(GRP * P // ck):
                    proj_ps = ppp.tile([P, ck], f32)
                    for m in range(ck // 512):
                        nc.tensor.matmul(
                            proj_ps[:, 512 * m:512 * (m + 1)],
                            wt_sb[:],
                            s["ht"][:, ck * c + 512 * m:ck * c + 512 * (m + 1)],
                            start=True, stop=True)
                    nc.scalar.activation(
                        tanh_g[:, ck * c:ck * (c + 1)], proj_ps[:],
                        mybir.ActivationFunctionType.Tanh, bias=bc_sb[:])
                s["tanh"] = tanh_g

            def stage_escore(it):
                s = st[it]
                e_ps = pep.tile([P, GRP], f32)
                for tt in range(GRP):
                    nc.tensor.matmul(
                        e_ps[:, tt:tt + 1],
                        s["tanh"][:, P * tt:P * (tt + 1)],
                        wb_sb[:], start=True, stop=True)
                a_sb = ap_.tile([P, GRP], f32, tag="a")
                nc.scalar.activation(
                    a_sb[:], e_ps[:], mybir.ActivationFunctionType.Exp,
                    bias=negc_sb[:])
                s["a"] = a_sb

            def stage_oh(it):
                s = st[it]
                ohs = []
                for tt in range(GRP):
                    t = s["g"] * GRP + tt
                    oh_sb = ohp.tile([P, w], f16, tag="oh")
                    nc.vector.tensor_scalar(
                        oh_sb[:], iota_sb[:], s["bl"][:, t:t + 1],
                        s["a"][:, tt:tt + 1],
                        mybir.AluOpType.is_equal, mybir.AluOpType.mult)
                    ohs.append(oh_sb)
                s["oh"] = ohs

            def stage_seg(it):
                s = st[it]
                # bank-aligned tile slots: a matmul may not straddle a bank
                spb = 512 // w
                nb = -(-GRP // spb)
                part_ps = psp.tile([P, GRP * w if nb == 1 else nb * 512], f32)
                for tt in range(GRP):
                    bb, sl = divmod(tt, spb)
                    c0 = bb * 512 + sl * w
                    nc.tensor.matmul(
                        part_ps[:, c0:c0 + w],
                        s["hseg"][:, tt], s["oh"][tt][:],
                        start=True, stop=True)
                part_sb = partp.tile([P, GRP * w], f32, tag="part")
                for bb in range(nb):
                    n_sl = min(spb, GRP - bb * spb)
                    nc.vector.tensor_copy(
                        part_sb[:, bb * spb * w:bb * spb * w + n_sl * w],
                        part_ps[:, bb * 512:bb * 512 + n_sl * w])
                nc.gpsimd.dma_start(part_d[s["i"], s["g"]], part_sb[:])
                nc.gpsimd.dma_start(av_d[s["i"], s["g"]], s["a"][:])
                st[it] = None

            for it in range(NITER + 3):
                if it < NITER:
                    stage_in(it)
                    stage_proj(it)
                if 0 <= it - 1 < NITER:
                    stage_escore(it - 1)
                if 0 <= it - 2 < NITER:
                    stage_oh(it - 2)
                if 0 <= it - 3 < NITER:
                    stage_seg(it - 3)
    nc.compile()
    return nc


def _prep_inputs(H, batch, W_proj, b_proj, w_score, cfg=CFG):
    """Host-side repack. Returns (in_maps, meta)."""
    V = H.shape[0]
    H = np.ascontiguousarray(H, dtype=np.float32)
    batch = np.asarray(batch).astype(np.int64)
    W_proj = np.asarray(W_proj, dtype=np.float32)
    b_proj = np.asarray(b_proj, dtype=np.float32)
    w_score = np.asarray(w_score, dtype=np.float32)

    c_shift = float(np.abs(w_score).sum())

    s = np.searchsorted(batch, np.arange(NBLK_TOT + 1, dtype=np.int64)
                        * SEGS_PER_BLK)
    lens = s[1:] - s[:-1]
    t_max = int(math.ceil(lens.max() / P))
    t_max = max(GRP, ((t_max + GRP - 1) // GRP) * GRP)

    tpos = np.arange(t_max * P, dtype=np.int64)
    idx = s[:NBLK_TOT, None] + tpos[None, :]              # [NBLK_TOT, t_max*P]
    # strict ownership: a node belongs to exactly one block (tiles of block
    # b may reach past its end; those spill nodes are zeroed/excluded here
    # and handled by block b+1)
    valid = tpos[None, :] < lens[:, None]
    idxc = np.minimum(idx, V - 1)

    segrel = (batch[idxc] - (np.arange(NBLK_TOT, dtype=np.int64)[:, None]
                             * SEGS_PER_BLK))             # [NBLK_TOT, t_max*P]
    segrel_t = segrel.reshape(NBLK_TOT, t_max, P)
    base = segrel_t[:, :, 0].copy()                       # [NBLK_TOT, t_max]
    base[~valid.reshape(NBLK_TOT, t_max, P)[:, :, 0]] = 0
    rel = segrel_t - base[:, :, None]
    w_data = int((np.where(valid.reshape(NBLK_TOT, t_max, P), rel, 0)).max()
                 + 1)
    w = max(cfg["w_min"], w_data)

    blv = rel.astype(np.float32)
    blv[~valid.reshape(NBLK_TOT, t_max, P)] = -1000.0
    # bl layout: [blk, P(node-in-tile), t_max]
    bl = np.ascontiguousarray(blv.transpose(0, 2, 1))

    NG = t_max // GRP
    Hg = H[idxc]                                          # [blk, t_max*P, D]
    Hg[~valid] = 0.0
    # hseg: [blk, NG, P(node), GRP*D]
    hseg5 = Hg.reshape(NBLK_TOT, NG, GRP, P, D)
    hseg = np.ascontiguousarray(
        hseg5.transpose(0, 1, 3, 2, 4).reshape(NBLK_TOT, NG, P, GRP * D)
        .astype(np.float16))
    # ht: [blk, NG, P(feature d), GRP*P(node)]
    ht = np.ascontiguousarray(
        hseg5.transpose(0, 1, 4, 2, 3).reshape(NBLK_TOT, NG, P, GRP * P)
        .astype(np.float16))
    del Hg, hseg5

    wt = np.ascontiguousarray(W_proj.T.astype(np.float16))      # [d, k]
    wb = np.ascontiguousarray(w_score.reshape(P, 1).astype(np.float16))
    bc = np.ascontiguousarray(b_proj.reshape(P, 1).astype(np.float32))
    iota = np.ascontiguousarray(
        np.broadcast_to(np.arange(w, dtype=np.float32), (P, w))
        .astype(np.float16))

    in_maps = []
    for c in range(N_CORES):
        sl = slice(c * NBLK, (c + 1) * NBLK)
        in_maps.append({
            "ht": np.ascontiguousarray(ht[sl]),
            "hseg": np.ascontiguousarray(hseg[sl]),
            "bl": np.ascontiguousarray(bl[sl]),
            "wt": wt, "wb": wb, "bc": bc, "iota": iota,
        })
    meta = {
        "t_max": t_max, "w": w, "c_shift": c_shift, "base": base,
        "valid": valid, "idxc": idxc, "batch": batch, "NG": NG,
    }
    return in_maps, meta


def _unshard(results, meta):
    """Host-side: scatter-add partials, denominator, divide."""
    t_max, w, NG = meta["t_max"], meta["w"], meta["NG"]
    base, valid, idxc, batch = (meta["base"], meta["valid"], meta["idxc"],
                                meta["batch"])

    # partials: per core [NBLK, NG, P(d), GRP*w] -> [NBLK_TOT, t_max, w, P(d)]
    part = np.concatenate([r["part"] for r in results], axis=0)
    part = (part.reshape(NBLK_TOT, NG, P, GRP, w)
            .transpose(0, 1, 3, 4, 2)
            .reshape(NBLK_TOT, t_max, w, P))
    # a values: per core [NBLK, NG, P(node), GRP] -> [NBLK_TOT, t_max*P]
    av = np.concatenate([r["av"] for r in results], axis=0)
    av = (av.reshape(NBLK_TOT, NG, P, GRP).transpose(0, 1, 3, 2)
          .reshape(NBLK_TOT, t_max * P))

    # match the f16 rounding the device applied to `a` inside the one-hot
    av16 = av.astype(np.float16).astype(np.float64)
    vm = valid
    den = np.bincount(batch[idxc][vm], weights=av16[vm],
                      minlength=G_SEGS)

    rows = (np.arange(NBLK_TOT, dtype=np.int64)[:, None, None] * SEGS_PER_BLK
            + base[:, :, None] + np.arange(w, dtype=np.int64)[None, None, :])
    np.clip(rows, 0, G_SEGS - 1, out=rows)
    out = np.zeros((G_SEGS, D), dtype=np.float64)
    np.add.at(out, rows.reshape(-1), part.reshape(-1, P).astype(np.float64))
    out /= np.maximum(den, 1e-12)[:, None]
    return out.astype(np.float32)


def kernel(H, batch, W_proj, b_proj, w_score):
    from concourse.bass_utils import run_bass_kernel_spmd

    in_maps, meta = _prep_inputs(H, batch, W_proj, b_proj, w_score, CFG)
    nc = _build_program(meta["t_max"], meta["w"], meta["c_shift"], CFG)
    res = run_bass_kernel_spmd(nc, in_maps, core_ids=list(range(N_CORES)))
    return _unshard(res.results, meta)


# revision 33
# speedup vs baseline: 1.0674x; 1.0674x over previous
"""AttentiveAggregation (segment softmax-pool) Trainium2 kernel, v2.

Math (per graph g): out_g = sum_v alpha_v H_v,  alpha = softmax_g(e),
  e_v = w_score . tanh(W_proj @ H_v + b_proj).

Device/host split (per 128-node tile, 8 cores x 16 blocks of 128 segments):
 * projT = W_proj @ H^T as one PE matmul per 512 nodes with the constant
   W_proj^T stationary; b_proj is a native per-partition ACT bias and the
   tanh runs on ACT in 512-wide chunks -> tanh_g [k, nodes] f16.
 * e-score on PE: lhsT = tanh tile [k, 128], rhs = w_score column -> e
   lands as [128 nodes, 1] PSUM, exactly the layout exp + the one-hot
   multiply need.  exp (with the global shift C = ||w_score||_1 plus an
   f16-range prescale) on ACT -> a [128, GRP] f32 in SBUF.
 * narrow one-hot on DVE: oh[n, j] = (seg_rel(n) == j) * a_n over a
   [128, W] window (W = max segments any tile spans, >= 8); seg_rel is
   tile-relative and streamed as data, so the program stays SPMD-uniform.
 * flipped segment matmul on PE: lhsT = H tile [node, d], rhs = oh ->
   partial [128 d, W] written at the tile's fixed PSUM slot (t*W).
   Partials and the a values are DMA'd out; the HOST does the final
   scatter-add over per-tile segment bases, the denominator bincount,
   and the division (cheap: ~16k x 128).
 * Streams: both input streams (ht = H^T per tile, hseg = H per tile) and
   bl ride the compute-free SP HWDGE queue (a compute engine's queue would
   delay DMA issue behind its semaphore waits); partials + a-values leave
   via the Pool SWDGE queue (idle engine, bypasses the shared HWDGE device),
   switching to SP for the pipeline-drain tail; constants load via Pool so
   SP's first instructions are the input streams.
 * A 3-stage software pipeline (proj/tanh -> escore/exp -> one-hot ->
   segmat/out) keeps every PE/ACT/DVE instruction's dependencies one full
   iteration old; sim shows DMA_ENGINES ~94% busy as the binding resource.
"""

import math

import numpy as np

P = 128                    # partitions / tile node count / D / HS
D = 128
G_SEGS = 16384
SEGS_PER_BLK = 128
NBLK_TOT = G_SEGS // SEGS_PER_BLK   # 128 global blocks
N_CORES = 8
NBLK = NBLK_TOT // N_CORES          # 16 blocks per core
GRP = 16                   # tiles per DMA group / pipeline stage

# alpha values are prescaled by exp(PRESCALE_LN) inside the exp bias so the
# per-segment weights stay in f16 normal range; numerator and denominator
# scale together so the final division cancels it exactly.
PRESCALE_LN = 14 * math.log(2.0)

CFG = {
    "proj_dt": "f16",      # f16 | f8 | f8x2 (fp8 H^T, two-level fp8 W)
    "w_min": 8,            # min one-hot width (keeps partial-DMA descs >=512B)
    "bufs_ht": 4,
    "bufs_hseg": 6,
    "bufs_tanh": 4,
    "bufs_oh": 36,
    "bufs_a": 4,
    "pp_proj": 2,
    "pp_e": 2,
    "pp_seg": 2,
    "tanh_chunk": 1024,
}


def _build_program(t_max, w, c_shift, cfg, repeat=1):
    import concourse.bacc as bacc
    import concourse.mybir as mybir
    import concourse.tile as tile

    f32 = mybir.dt.float32
    f16 = mybir.dt.float16
    NG = t_max // GRP
    NITER = NBLK * NG          # pipeline iterations (one per tile group)

    # fit the three PSUM pools into the 8 banks (2KB each per partition)
    cfg = dict(cfg)

    def banks():
        bp = -(-4 * cfg["tanh_chunk"] // 2048) * cfg["pp_proj"]
        be = cfg["pp_e"]
        bs = -(-GRP // (512 // w)) * cfg["pp_seg"]
        return bp + be + bs

    for knob, val in (("tanh_chunk", 512), ("pp_seg", 1), ("pp_proj", 1)):
        if banks() > 8:
            cfg[knob] = val
    assert banks() <= 8, f"PSUM over budget: {banks()} banks (w={w})"

    f8 = mybir.dt.float8e4
    proj_dt = cfg.get("proj_dt", "f16")
    dt_ht = f8 if proj_dt in ("f8", "f8x2") else f16

    nc = bacc.Bacc(None, target_bir_lowering=False)
    ht_d = nc.dram_tensor("ht", [NBLK, NG, P, GRP * P], dt_ht,
                          kind="ExternalInput")
    hseg_d = nc.dram_tensor("hseg", [NBLK, NG, P, GRP * P], f16,
                            kind="ExternalInput")
    bl_d = nc.dram_tensor("bl", [NBLK, P, t_max], f32, kind="ExternalInput")
    wt_d = nc.dram_tensor("wt", [P, D], dt_ht, kind="ExternalInput")
    if proj_dt == "f8x2":
        wtc_d = nc.dram_tensor("wtc", [P, D], f8, kind="ExternalInput")
    wb_d = nc.dram_tensor("wb", [P, 1], f16, kind="ExternalInput")
    bc_d = nc.dram_tensor("bc", [P, 1], f32, kind="ExternalInput")
    iota_d = nc.dram_tensor("iota", [P, w], f16, kind="ExternalInput")
    part_d = nc.dram_tensor("part", [NBLK, NG, P, GRP * w], f32,
                            kind="ExternalOutput")
    av_d = nc.dram_tensor("av", [NBLK, NG, P, GRP], f32,
                          kind="ExternalOutput")

    with tile.TileContext(nc) as tc:
        with (
            tc.tile_pool(name="const", bufs=1) as constp,
            tc.tile_pool(name="htp", bufs=cfg["bufs_ht"]) as htp,
            tc.tile_pool(name="hsegp", bufs=cfg["bufs_hseg"]) as hsegp,
            tc.tile_pool(name="tanhp", bufs=cfg["bufs_tanh"]) as tanhp,
            tc.tile_pool(name="ohp", bufs=cfg["bufs_oh"]) as ohp,
            tc.tile_pool(name="ap_", bufs=cfg["bufs_a"]) as ap_,
            tc.tile_pool(name="blp", bufs=4) as blp,
            tc.tile_pool(name="partp", bufs=3) as partp,
            tc.tile_pool(name="ppp", bufs=cfg["pp_proj"],
                         space="PSUM") as ppp,
            tc.tile_pool(name="pep", bufs=cfg["pp_e"], space="PSUM") as pep,
            tc.tile_pool(name="psp", bufs=cfg["pp_seg"], space="PSUM") as psp,
        ):
            # consts go on the Pool SWDGE queue so the SP queue's first
            # instructions are the input streams
            wt_sb = constp.tile([P, D], dt_ht)
            nc.gpsimd.dma_start(wt_sb[:], wt_d[:])
            if proj_dt == "f8x2":
                wtc_sb = constp.tile([P, D], f8)
                nc.gpsimd.dma_start(wtc_sb[:], wtc_d[:])
            wb_sb = constp.tile([P, 1], f16)
            nc.gpsimd.dma_start(wb_sb[:], wb_d[:])
            bc_sb = constp.tile([P, 1], f32)
            nc.gpsimd.dma_start(bc_sb[:], bc_d[:])
            iota_sb = constp.tile([P, w], f16)
            nc.gpsimd.dma_start(iota_sb[:], iota_d[:])
            negc_sb = constp.tile([P, 1], f32)
            nc.gpsimd.memset(negc_sb[:], -float(c_shift) + PRESCALE_LN)

            # per-iteration state carried across pipeline stages
            st = [None] * NITER
            bl_tiles = [None] * NBLK

            def stage_in(it):
                i, g = divmod(it, NG)
                # prefetch each block's bl one group ahead of first use
                ip = (it + 1) // NG
                if it == 0 or (it + 1) % NG == 0:
                    ib = 0 if it == 0 else ip
                    if ib < NBLK and bl_tiles[ib] is None:
                        bl_sb = blp.tile([P, t_max], f32)
                        nc.sync.dma_start(bl_sb[:], bl_d[ib])
                        bl_tiles[ib] = bl_sb
                ht_sb = htp.tile([P, GRP * P], dt_ht)
                nc.sync.dma_start(ht_sb[:], ht_d[i, g])
                hseg_sb = hsegp.tile([P, GRP, P], f16)
                nc.sync.dma_start(
                    hseg_sb[:],
                    hseg_d[i, g].rearrange("p (t c) -> p t c", t=GRP))
                st[it] = {"ht": ht_sb, "hseg": hseg_sb, "bl": bl_tiles[i],
                          "i": i, "g": g}

            def stage_proj(it):
                s = st[it]
                tanh_g = tanhp.tile([P, GRP * P], f16, tag="tanh")
                ck = cfg["tanh_chunk"]
                for c in range(GRP * P // ck):
                    proj_ps = ppp.tile([P, ck], f32)
                    for m in range(ck // 512):
                        sl = s["ht"][:, ck * c + 512 * m:ck * c + 512 * (m + 1)]
                        if proj_dt == "f8x2":
                            nc.tensor.matmul(
                                proj_ps[:, 512 * m:512 * (m + 1)],
                                wt_sb[:], sl, start=True, stop=False)
                            nc.tensor.matmul(
                                proj_ps[:, 512 * m:512 * (m + 1)],
                                wtc_sb[:], sl, start=False, stop=True)
                        else:
                            nc.tensor.matmul(
                                proj_ps[:, 512 * m:512 * (m + 1)],
                                wt_sb[:], sl, start=True, stop=True)
                    nc.scalar.activation(
                        tanh_g[:, ck * c:ck * (c + 1)], proj_ps[:],
                        mybir.ActivationFunctionType.Tanh, bias=bc_sb[:])
                s["tanh"] = tanh_g

            def stage_escore(it):
                s = st[it]
                e_ps = pep.tile([P, GRP], f32)
                for tt in range(GRP):
                    nc.tensor.matmul(
                        e_ps[:, tt:tt + 1],
                        s["tanh"][:, P * tt:P * (tt + 1)],
                        wb_sb[:], start=True, stop=True)
                a_sb = ap_.tile([P, GRP], f32, tag="a")
                nc.scalar.activation(
                    a_sb[:], e_ps[:], mybir.ActivationFunctionType.Exp,
                    bias=negc_sb[:])
                s["a"] = a_sb

            def stage_oh(it):
                s = st[it]
                ohs = []
                for tt in range(GRP):
                    t = s["g"] * GRP + tt
                    oh_sb = ohp.tile([P, w], f16, tag="oh")
                    nc.vector.tensor_scalar(
                        oh_sb[:], iota_sb[:], s["bl"][:, t:t + 1],
                        s["a"][:, tt:tt + 1],
                        mybir.AluOpType.is_equal, mybir.AluOpType.mult)
                    ohs.append(oh_sb)
                s["oh"] = ohs

            def stage_seg(it):
                s = st[it]
                # bank-aligned tile slots: a matmul may not straddle a bank
                spb = 512 // w
                nb = -(-GRP // spb)
                part_ps = psp.tile([P, GRP * w if nb == 1 else nb * 512], f32)
                for tt in range(GRP):
                    bb, sl = divmod(tt, spb)
                    c0 = bb * 512 + sl * w
                    nc.tensor.matmul(
                        part_ps[:, c0:c0 + w],
                        s["hseg"][:, tt], s["oh"][tt][:],
                        start=True, stop=True)
                part_sb = partp.tile([P, GRP * w], f32, tag="part")
                for bb in range(nb):
                    n_sl = min(spb, GRP - bb * spb)
                    nc.vector.tensor_copy(
                        part_sb[:, bb * spb * w:bb * spb * w + n_sl * w],
                        part_ps[:, bb * 512:bb * 512 + n_sl * w])
                # tail iterations: input streams are done, SP is idle
                out_eng = nc.sync if it >= NITER - 4 else nc.gpsimd
                out_eng.dma_start(part_d[s["i"], s["g"]], part_sb[:])
                out_eng.dma_start(av_d[s["i"], s["g"]], s["a"][:])
                st[it] = None

            def emit_all():
                for it in range(NITER + 3):
                    if it < NITER:
                        stage_in(it)
                        stage_proj(it)
                    if 0 <= it - 1 < NITER:
                        stage_escore(it - 1)
                    if 0 <= it - 2 < NITER:
                        stage_oh(it - 2)
                    if 0 <= it - 3 < NITER:
                        stage_seg(it - 3)

            if repeat > 1:
                with tc.For_i(0, repeat, 1):
                    emit_all()
            else:
                emit_all()
    nc.compile()
    return nc


def _prep_inputs(H, batch, W_proj, b_proj, w_score, cfg=CFG):
    """Host-side repack. Returns (in_maps, meta)."""
    V = H.shape[0]
    H = np.ascontiguousarray(H, dtype=np.float32)
    batch = np.asarray(batch).astype(np.int64)
    W_proj = np.asarray(W_proj, dtype=np.float32)
    b_proj = np.asarray(b_proj, dtype=np.float32)
    w_score = np.asarray(w_score, dtype=np.float32)

    c_shift = float(np.abs(w_score).sum())

    s = np.searchsorted(batch, np.arange(NBLK_TOT + 1, dtype=np.int64)
                        * SEGS_PER_BLK)
    lens = s[1:] - s[:-1]
    t_max = int(math.ceil(lens.max() / P))
    t_max = max(GRP, ((t_max + GRP - 1) // GRP) * GRP)

    tpos = np.arange(t_max * P, dtype=np.int64)
    idx = s[:NBLK_TOT, None] + tpos[None, :]              # [NBLK_TOT, t_max*P]
    # strict ownership: a node belongs to exactly one block (tiles of block
    # b may reach past its end; those spill nodes are zeroed/excluded here
    # and handled by block b+1)
    valid = tpos[None, :] < lens[:, None]
    idxc = np.minimum(idx, V - 1)

    segrel = (batch[idxc] - (np.arange(NBLK_TOT, dtype=np.int64)[:, None]
                             * SEGS_PER_BLK))             # [NBLK_TOT, t_max*P]
    segrel_t = segrel.reshape(NBLK_TOT, t_max, P)
    base = segrel_t[:, :, 0].copy()                       # [NBLK_TOT, t_max]
    base[~valid.reshape(NBLK_TOT, t_max, P)[:, :, 0]] = 0
    rel = segrel_t - base[:, :, None]
    w_data = int((np.where(valid.reshape(NBLK_TOT, t_max, P), rel, 0)).max()
                 + 1)
    w = max(cfg["w_min"], w_data)

    blv = rel.astype(np.float32)
    blv[~valid.reshape(NBLK_TOT, t_max, P)] = -1000.0
    # bl layout: [blk, P(node-in-tile), t_max]
    bl = np.ascontiguousarray(blv.transpose(0, 2, 1))

    proj_dt = cfg.get("proj_dt", "f16")
    if proj_dt in ("f8", "f8x2"):
        import ml_dtypes
        np_ht = ml_dtypes.float8_e4m3fn
    else:
        np_ht = np.float16

    NG = t_max // GRP
    Hg = H[idxc]                                          # [blk, t_max*P, D]
    Hg[~valid] = 0.0
    # hseg: [blk, NG, P(node), GRP*D]
    hseg5 = Hg.reshape(NBLK_TOT, NG, GRP, P, D)
    hseg = np.ascontiguousarray(
        hseg5.transpose(0, 1, 3, 2, 4).reshape(NBLK_TOT, NG, P, GRP * D)
        .astype(np.float16))
    # ht: [blk, NG, P(feature d), GRP*P(node)]
    ht = np.ascontiguousarray(
        hseg5.transpose(0, 1, 4, 2, 3).reshape(NBLK_TOT, NG, P, GRP * P)
        .astype(np_ht))
    del Hg, hseg5

    wtf = W_proj.T.astype(np.float32)                           # [d, k]
    wt = np.ascontiguousarray(wtf.astype(np_ht))
    wtc = np.ascontiguousarray(
        (wtf - wt.astype(np.float32)).astype(np_ht))
    wb = np.ascontiguousarray(w_score.reshape(P, 1).astype(np.float16))
    bc = np.ascontiguousarray(b_proj.reshape(P, 1).astype(np.float32))
    iota = np.ascontiguousarray(
        np.broadcast_to(np.arange(w, dtype=np.float32), (P, w))
        .astype(np.float16))

    in_maps = []
    for c in range(N_CORES):
        sl = slice(c * NBLK, (c + 1) * NBLK)
        m = {
            "ht": np.ascontiguousarray(ht[sl]),
            "hseg": np.ascontiguousarray(hseg[sl]),
            "bl": np.ascontiguousarray(bl[sl]),
            "wt": wt, "wb": wb, "bc": bc, "iota": iota,
        }
        if proj_dt == "f8x2":
            m["wtc"] = wtc
        in_maps.append(m)
    meta = {
        "t_max": t_max, "w": w, "c_shift": c_shift, "base": base,
        "valid": valid, "idxc": idxc, "batch": batch, "NG": NG,
    }
    return in_maps, meta


def _unshard(results, meta):
    """Host-side: scatter-add partials, denominator, divide."""
    t_max, w, NG = meta["t_max"], meta["w"], meta["NG"]
    base, valid, idxc, batch = (meta["base"], meta["valid"], meta["idxc"],
                                meta["batch"])

    # partials: per core [NBLK, NG, P(d), GRP*w] -> [NBLK_TOT, t_max, w, P(d)]
    part = np.concatenate([r["part"] for r in results], axis=0)
    part = (part.reshape(NBLK_TOT, NG, P, GRP, w)
            .transpose(0, 1, 3, 4, 2)
            .reshape(NBLK_TOT, t_max, w, P))
    # a values: per core [NBLK, NG, P(node), GRP] -> [NBLK_TOT, t_max*P]
    av = np.concatenate([r["av"] for r in results], axis=0)
    av = (av.reshape(NBLK_TOT, NG, P, GRP).transpose(0, 1, 3, 2)
          .reshape(NBLK_TOT, t_max * P))

    # match the f16 rounding the device applied to `a` inside the one-hot
    av16 = av.astype(np.float16).astype(np.float64)
    vm = valid
    den = np.bincount(batch[idxc][vm], weights=av16[vm],
                      minlength=G_SEGS)

    rows = (np.arange(NBLK_TOT, dtype=np.int64)[:, None, None] * SEGS_PER_BLK
            + base[:, :, None] + np.arange(w, dtype=np.int64)[None, None, :])
    np.clip(rows, 0, G_SEGS - 1, out=rows)
    out = np.zeros((G_SEGS, D), dtype=np.float64)
    np.add.at(out, rows.reshape(-1), part.reshape(-1, P).astype(np.float64))
    out /= np.maximum(den, 1e-12)[:, None]
    return out.astype(np.float32)


def kernel(H, batch, W_proj, b_proj, w_score):
    from concourse.bass_utils import run_bass_kernel_spmd

    in_maps, meta = _prep_inputs(H, batch, W_proj, b_proj, w_score, CFG)
    nc = _build_program(meta["t_max"], meta["w"], meta["c_shift"], CFG)
    res = run_bass_kernel_spmd(nc, in_maps, core_ids=list(range(N_CORES)))
    return _unshard(res.results, meta)
